# revision 44
# baseline (speedup 1.0000x reference)
"""Trainium2 Bass kernel for CustomMamba (data-parallel over (b*n) scans).

Self-contained: builds + compiles a single-core SPMD Bass/Tile program,
shards inputs over 8 NeuronCores (along n), runs via run_bass_kernel_spmd,
and gathers the full output.

v3 highlights (2.4x faster than the fp32 baseline in the cost model):
- fp16 compute path end to end (all values here are O(1)); the scan
  recurrence state stays fp32 inside tensor_tensor_scan, and the
  softplus exp intermediate uses bf16 for range.
- The depthwise causal conv is folded into M1 as four PSUM-accumulated
  matmuls with host-prescaled weights (W1x * conv_w[:,k]) and t-shifted
  moving operands.
- The s-sum of C_s * h_s is accumulated on the PE via identity-matmul
  PSUM accumulation instead of a DVE add chain.
- Software-pipelined emission: each block's front-end is emitted as
  ~50 small thunks interleaved into the previous block's 32 scan
  iterations, so the in-order per-engine queues overlap blocks.
- Engine balance: scans + most elementwise on DVE, exps + PSUM copies
  on Act, a tunable share of the broadcast mults on GPSIMD, transposes
  + y-accumulation on PE. One batched 786KB output DMA per block.
"""

import sys

sys.path.insert(0, "/opt/trn_rl_repo")

import os

os.environ.setdefault("JAX_PLATFORMS", "cpu")

from contextlib import ExitStack

import numpy as np

import concourse.bacc as bacc
import concourse.mybir as mybir
from concourse.bass_utils import run_bass_kernel_spmd
from concourse.masks import make_identity
from concourse.tile import TileContext
from concourse._compat import axon_active

FP = mybir.dt.float32
BF = mybir.dt.float16   # compute dtype (fp16: all values here are O(1))
HF = mybir.dt.float16
B16 = mybir.dt.bfloat16  # wide-range dtype for the softplus exp intermediate
AF = mybir.ActivationFunctionType
OP = mybir.AluOpType

# Problem constants (hardcoded per spec)
B, T, N, F = 8, 24, 512, 64
DI, DS, DR, DC = 128, 16, 4, 4
NCORES = 8


def _host_consts(inputs):
    """Fold the linear layers into per-stage weight matrices (fp32 numpy)."""
    w_mix = np.asarray(inputs["w_mix"], np.float32)      # [2F, F]
    b_mix = np.asarray(inputs["b_mix"], np.float32)      # [F]
    w_in = np.asarray(inputs["w_in"], np.float32)        # [F, 2*DI]
    conv_w = np.asarray(inputs["conv_w"], np.float32)    # [DI, DC]
    conv_b = np.asarray(inputs["conv_b"], np.float32)    # [DI]
    w_xproj = np.asarray(inputs["w_xproj"], np.float32)  # [DI, DR+2*DS]
    w_dt = np.asarray(inputs["w_dt"], np.float32)        # [DR, DI]
    b_dt = np.asarray(inputs["b_dt"], np.float32)        # [DI]
    A_log = np.asarray(inputs["A_log"], np.float32)      # [DI, DS]
    D = np.asarray(inputs["D"], np.float32)              # [DI]
    w_out = np.asarray(inputs["w_out"], np.float32)      # [DI, F]

    W1 = w_mix @ w_in                                    # [128, 2*DI]
    b1 = b_mix @ w_in                                    # [2*DI]
    W1x, W1z = W1[:, :DI].copy(), W1[:, DI:].copy()
    b1x, b1z = b1[:DI].copy(), b1[DI:].copy()

    W2dt = (w_xproj[:, :DR] @ w_dt).copy()               # [DI, DI]
    W2bc = w_xproj[:, DR:].copy()                        # [DI, 2*DS]

    A = -np.exp(A_log)                                   # [DI, DS]
    assert np.allclose(A, A[0:1, :], rtol=1e-6), "A varies across d"
    A_s = [float(A[0, s]) for s in range(DS)]

    # fold the depthwise conv into M1: per-tap column-scaled weights
    W1xk = [np.ascontiguousarray(W1x * conv_w[None, :, k]) for k in range(DC)]

    return dict(
        W1x=W1x, W1z=W1z, b1x=b1x, b1z=b1z, W1xk=W1xk,
        W2dt=W2dt, W2bc=W2bc, b_dt=b_dt,
        conv_w=conv_w, conv_b=conv_b, D=D, w_out=w_out, A_s=A_s,
    )


def build_program(n_c, consts, debug=None, **tune):
    """Build + compile the per-core Bass program. n_c = n-shard width."""
    if debug is None:
        debug = not axon_active()
    nc = bacc.Bacc(
        "TRN2",
        target_bir_lowering=False,
        debug=debug,
        enable_asserts=True,
        num_devices=1,
    )

    bn = B * n_c
    ic = min(128, bn)
    nblk = bn // ic
    assert nblk * ic == bn
    bpb = ic // n_c                    # b's per block
    assert bpb * n_c == ic and bpb >= 1
    CT = ic * T

    x_d = nc.dram_tensor("x_sh", (B, T, n_c, F), FP, kind="ExternalInput").ap()
    qk_d = nc.dram_tensor("qk_sh", (B, T, n_c, F), FP, kind="ExternalInput").ap()
    cd = {}
    for nm, shp in [
        ("W1xk0", (2 * F, DI)), ("W1xk1", (2 * F, DI)),
        ("W1xk2", (2 * F, DI)), ("W1xk3", (2 * F, DI)), ("W1z", (2 * F, DI)),
        ("b1x", (DI, 1)), ("b1z", (DI, 1)),
        ("W2dt", (DI, DI)), ("W2bc", (DI, 2 * DS)), ("b_dt", (DI, 1)),
        ("conv_w", (DI, DC)), ("conv_b", (DI, 1)), ("D", (DI, 1)),
        ("w_out", (DI, F)),
    ]:
        cd[nm] = nc.dram_tensor(nm, shp, FP, kind="ExternalInput").ap()
    out_d = nc.dram_tensor("out_sh", (B, T, n_c, F), FP, kind="ExternalOutput").ap()

    with TileContext(nc) as tc:
        _body(nc, tc, x_d, qk_d, cd, out_d, n_c, ic, nblk, bpb, CT, consts,
              tune)
    nc.compile()
    return nc


def _body(nc, tc, x_d, qk_d, cd, out_d, n_c, ic, nblk, bpb, CT, consts, tune):
    P = ic
    DH = 64                            # d-half width for scan-phase tiles
    NDH = DI // DH
    NMM = 512 if CT % 512 == 0 else CT  # matmul N-chunk
    TG = 8                             # t's merged per transpose-psum tile
    use_b1 = not (np.allclose(consts["b1x"], 0) and np.allclose(consts["b1z"], 0))
    use_cb = not np.allclose(consts["conv_b"], 0)
    use_d = not np.allclose(consts["D"], 1.0)
    A_s = consts["A_s"]
    XM_POOL = tune.get("xm_pool", 3)   # xmult on Pool when s %% this == 0
    CM_POOL = tune.get("cm_pool", 3)   # cmul on Pool when s %% this == off
    CM_OFF = tune.get("cm_off", 0)
    XM_OFF = tune.get("xm_off", 0)
    PACE_N, PACE_D = tune.get("pace", (1, 1))  # front-interleave speed
    IN_DVE = tune.get("in_dve", False)  # odd input-copy halves on DVE
    YA_DVE = tune.get("ya_dve", False)  # ya psum->sbuf copy on DVE

    es = ExitStack()
    cb = es.enter_context(tc.tile_pool(name="cb", bufs=1))    # constants
    sb = es.enter_context(tc.tile_pool(name="sb", bufs=2))    # block-rotating
    sb1 = es.enter_context(tc.tile_pool(name="sb1", bufs=1))  # out staging
    SB2B = tune.get("sb2b", 4)
    sb2 = es.enter_context(tc.tile_pool(name="sb2", bufs=SB2B))  # scan temps
    ps = es.enter_context(tc.tile_pool(name="ps", bufs=2, space="PSUM"))
    ps1 = es.enter_context(tc.tile_pool(name="ps1", bufs=1, space="PSUM"))

    # ---- constants: load fp32, cast matmul weights to bf16 on-chip ----
    ctf = {}
    for nm in cd:
        t = cb.tile(list(cd[nm].shape), FP, tag=f"cf_{nm}")
        nc.sync.dma_start(t[:], cd[nm])
        ctf[nm] = t
    ct = dict(ctf)
    for nm in ("W1xk0", "W1xk1", "W1xk2", "W1xk3", "W1z", "W2dt", "W2bc", "w_out"):
        tb = cb.tile(list(cd[nm].shape), BF, tag=f"cb_{nm}")
        nc.scalar.copy(out=tb[:], in_=ctf[nm][:])
        ct[nm] = tb
    ident = cb.tile([128, 128], FP, tag="ident")
    make_identity(nc, ident[:])
    identb = cb.tile([128, 128], BF, tag="identb")
    make_identity(nc, identb[:])
    identh = cb.tile([128, 128], HF, tag="identh")
    make_identity(nc, identh[:])

    assert bpb * T <= 64
    nrow = bpb * T
    slot = 64
    NH = 4                             # n-chunks per raw load
    nhw = n_c // NH
    NP = 8                             # n-pairs per transpose-psum tile
    assert nhw == 2 * NP
    da_zeroed = [0]

    def front_units(blk):
        """Block blk's front-end as emission thunks, in dependency order.
        Interleaved into the previous block's scan phase so every engine's
        in-order queue alternates between the two blocks."""
        b0 = blk * bpb
        st = {}
        units = []

        # -- load + transpose x/qk into xcatT [128=(fx|fqk), (i,t)] bf16;
        #    scan index i = (n, b) so out-phase transposes see contiguous
        #    (b,t) slabs per n --
        def mk_load(src_d, half, tagr, nh):
            def u():
                if "xcatT" not in st:
                    st["xcatT"] = sb.tile([128, CT], BF, tag="xcatT", name="xcatT")
                nb = nh * nhw
                raw = sb.tile([nrow, nhw * F], FP, tag=tagr, name="raw")
                nc.sync.dma_start(
                    raw[:],
                    src_d[b0:b0 + bpb, :, nb:nb + nhw].rearrange(
                        "b t n f -> (b t) (n f)"),
                )
                pt = ps.tile([2 * F, NP * slot], FP, tag="tt")
                for k in range(NP):
                    nc.tensor.transpose(
                        pt[:, k * slot:k * slot + nrow],
                        raw[:, 2 * k * F:2 * (k + 1) * F],
                        ident[:nrow, :nrow],
                    )
                xv = st["xcatT"][half * F:(half + 1) * F, :].rearrange(
                    "p (i t) -> p i t", t=T
                ).rearrange("p (n b) t -> p n b t", b=bpb)
                for par in range(2):
                    dst = xv[:, nb + par:nb + 2 * NP:2, :]
                    src_ap = pt[par * F:(par + 1) * F, :].rearrange(
                        "p (n r) -> p n r", r=slot)[:, :, :nrow].rearrange(
                        "p n (b t) -> p n b t", t=T)
                    if blk == 0 or (IN_DVE and par == 1):
                        # head fill: block 0 has nothing to overlap, so
                        # put its input copies on the otherwise-idle DVE
                        nc.vector.tensor_copy(out=dst, in_=src_ap)
                    else:
                        nc.scalar.copy(out=dst, in_=src_ap)
            return u
        for src_d, half, tagr in ((x_d, 0, "xraw"), (qk_d, 1, "qraw")):
            for nh in range(NH):
                units.append(mk_load(src_d, half, tagr, nh))

        # -- M1: z = W1z.T @ xcatT; conv-folded xc2 preact via 4
        #    accumulated matmuls with t-shifted moving operands --
        def mk_m1z(c0):
            def u():
                if "z" not in st:
                    st["z"] = sb.tile([DI, CT], BF, tag="z", name="z")
                pz = ps1.tile([DI, NMM], FP, tag="m1b")
                nc.tensor.matmul(pz[:], ct["W1z"][:],
                                 st["xcatT"][:, c0:c0 + NMM],
                                 start=True, stop=True)
                if use_b1:
                    nc.scalar.activation(st["z"][:, c0:c0 + NMM], pz[:],
                                         AF.Identity, bias=ctf["b1z"][:, 0:1])
                else:
                    nc.scalar.copy(out=st["z"][:, c0:c0 + NMM], in_=pz[:])
            return u
        for c0 in range(0, CT, NMM):
            units.append(mk_m1z(c0))

        ICH = 16                       # i's per conv-fold psum chunk
        NCC = ICH * T                  # 384 cols
        def mk_m1conv(i0):
            def u():
                if "acc" not in st:
                    st["acc"] = sb.tile([DI, CT], BF, tag="acc", name="acc")
                x3 = st["xcatT"][:].rearrange("p (i t) -> p i t", t=T)
                pxc = ps.tile([DI, NCC], FP, tag="m1a")
                p3 = pxc[:].rearrange("p (i t) -> p i t", t=T)
                nc.tensor.matmul(pxc[:], ct["W1xk3"][:],
                                 st["xcatT"][:, i0 * T:i0 * T + NCC],
                                 start=True, stop=False)
                for k in range(DC - 1):
                    d = DC - 1 - k
                    nc.tensor.matmul(
                        p3[:, :, d:], ct[f"W1xk{k}"][:],
                        x3[:, i0:i0 + ICH, :T - d],
                        start=False, stop=(k == DC - 2))
                dst = st["acc"][:, i0 * T:i0 * T + NCC]
                if use_cb:
                    nc.scalar.activation(dst, pxc[:], AF.Identity,
                                         bias=ctf["conv_b"][:, 0:1])
                else:
                    nc.scalar.copy(out=dst, in_=pxc[:])
            return u
        for i0 in range(0, P, ICH):
            units.append(mk_m1conv(i0))

        def u_silu():
            acc = st["acc"]
            sg = sb.tile([DI, CT], BF, tag="bc")
            nc.scalar.activation(sg[:], acc[:], AF.Sigmoid)
            nc.vector.tensor_tensor(acc[:], acc[:], sg[:], OP.mult)
        units.append(u_silu)

        # -- M2: dt = softplus(W2dt.T @ xc2 + b_dt); bc = W2bc.T @ xc2 --
        def mk_m2(c0):
            def u():
                if "dt" not in st:
                    st["dt"] = sb.tile([DI, CT], HF, tag="dt_y", name="dt")
                    st["bc"] = sb.tile([2 * DS, CT], BF, tag="bc", name="bc")
                xc2 = st["acc"]
                pdt = ps.tile([DI, NMM], FP, tag="m1a")
                pbc = ps1.tile([2 * DS, NMM], FP, tag="m1b")
                nc.tensor.matmul(pdt[:], ct["W2dt"][:], xc2[:, c0:c0 + NMM],
                                 start=True, stop=True)
                nc.tensor.matmul(pbc[:], ct["W2bc"][:], xc2[:, c0:c0 + NMM],
                                 start=True, stop=True)
                # softplus = ln(1+exp(.)); Ln deferred full-width to avoid
                # per-chunk act-table thrash between Exp and Ln. The exp
                # intermediate must be bf16: e^x can overflow fp16 range.
                if "spe" not in st:
                    st["spe"] = sb1.tile([DI, CT], B16, tag="spe", name="spe")
                nc.scalar.activation(st["spe"][:, c0:c0 + NMM], pdt[:], AF.Exp,
                                     bias=ctf["b_dt"][:, 0:1])
                if tune.get("bc_act", False):
                    nc.scalar.copy(out=st["bc"][:, c0:c0 + NMM], in_=pbc[:])
                else:
                    nc.vector.tensor_copy(out=st["bc"][:, c0:c0 + NMM],
                                          in_=pbc[:])
            return u
        for c0 in range(0, CT, NMM):
            units.append(mk_m2(c0))

        def u_ln():
            nc.scalar.activation(st["dt"][:], st["spe"][:], AF.Ln, bias=1.0)
        units.append(u_ln)

        def u_du():
            st["du"] = sb.tile([DI, CT], BF, tag="du_sz", name="du")
            nc.vector.tensor_tensor(st["du"][:], st["dt"][:], st["acc"][:],
                                    OP.mult)
        units.append(u_du)

        # -- transpose dt,du -> [i,(d,t)]; bc -> [i,(sc,t)] (bf16) --
        def mk_t(srckey, dstkey, tag, rows, use_act, t0):
            def u():
                dty = st[srckey].dtype
                idt = identh if dty == HF else identb
                if dstkey not in st:
                    st[dstkey] = sb.tile(
                        [P, rows * T], dty, tag=tag, name=dstkey)
                s3 = st[srckey][:].rearrange("p (i t) -> p i t", t=T)
                pt = ps.tile([P, TG * rows], dty, tag="tt")
                for k in range(TG):
                    nc.tensor.transpose(
                        pt[:, k * rows:(k + 1) * rows],
                        s3[:rows, :, t0 + k],
                        idt[:rows, :rows],
                    )
                dst = st[dstkey][:].rearrange(
                    "p (d t) -> p d t", t=T)[:, :, t0:t0 + TG]
                src_ap = pt[:].rearrange("p (t d) -> p d t", t=TG)
                if use_act:
                    nc.scalar.copy(out=dst, in_=src_ap)
                else:
                    nc.vector.tensor_copy(out=dst, in_=src_ap)
            return u
        for srckey, dstkey, tag, rows, use_act in (
                ("dt", "dtT", "dtT", DI, True),
                ("du", "duT", "duT", DI, tune.get("dut_act", False)),
                ("bc", "bcT", "bcT", 2 * DS, True)):
            for t0 in range(0, T, TG):
                units.append(mk_t(srckey, dstkey, tag, rows, use_act, t0))
        return units, st

    def scanback(blk, st, extra):
        """Scan phase + gate + M3 + out for block blk; interleaves `extra`
        (the next block's front-end thunks) across the scan iterations."""
        b0 = blk * bpb
        y_d = sb.tile([DI, CT], BF, tag="dt_y")  # dt dead post-transpose
        dtT, duT, bcT = st["dtT"], st["duT"], st["bcT"]
        duT3 = duT[:].rearrange("p (d t) -> p d t", t=T)
        bcT3 = bcT[:].rearrange("p (c t) -> p c t", t=T)
        ndone = 0
        nit = NDH * DS
        for dh in range(NDH):
            d0 = dh * DH
            dtv = dtT[:, d0 * T:(d0 + DH) * T].rearrange(
                "p (d t) -> p d t", t=T)
            for s in range(DS):
                # dA fp16; t=0 column is zeroed once per physical buffer and
                # the exp writes only t>=1 -> scan state resets per segment.
                dA = sb2.tile([P, DH * T], HF, tag="dA")
                dA3 = dA[:].rearrange("p (d t) -> p d t", t=T)
                if da_zeroed[0] < SB2B:
                    nc.gpsimd.memset(dA3[:, :, 0:1], 0.0)
                    da_zeroed[0] += 1
                nc.scalar.activation(dA3[:, :, 1:], dtv[:, :, 1:],
                                     AF.Exp, scale=A_s[s])
                Xs = sb2.tile([P, DH * T], HF, tag="Xs")
                xe = nc.gpsimd if (s % XM_POOL) == XM_OFF else nc.vector
                xe.tensor_tensor(
                    Xs[:].rearrange("p (d t) -> p d t", t=T),
                    duT3[:, d0:d0 + DH],
                    bcT3[:, s:s + 1, :].to_broadcast((P, DH, T)),
                    OP.mult,
                )
                hs = sb2.tile([P, DH * T], HF, tag="hs")
                nc.vector.tensor_tensor_scan(hs[:], dA[:], Xs[:], 0.0,
                                             OP.mult, OP.add)
                tmp = sb2.tile([P, DH * T], HF, tag="tmp")
                ce = nc.gpsimd if (s % CM_POOL) == CM_OFF else nc.vector
                ce.tensor_tensor(
                    tmp[:].rearrange("p (d t) -> p d t", t=T),
                    hs[:].rearrange("p (d t) -> p d t", t=T),
                    bcT3[:, DS + s:DS + s + 1, :].to_broadcast((P, DH, T)),
                    OP.mult,
                )
                # accumulate sum_s tmp_s on the PE into PSUM (identity
                # matmul, start on s==0): no DVE adds needed
                if s == 0:
                    yacc = ps1.tile([P, DH * T], FP, tag="yacc")
                for c0m in range(0, DH * T, NMM):
                    nc.tensor.matmul(yacc[:, c0m:c0m + NMM], identb[:P, :P],
                                     tmp[:, c0m:c0m + NMM],
                                     start=(s == 0), stop=(s == DS - 1))
                # interleave next block's front-end
                it = dh * DS + s + 1
                want = min(len(extra),
                           len(extra) * it * PACE_N // (nit * PACE_D))
                while ndone < want:
                    extra[ndone]()
                    ndone += 1
            ya = sb2.tile([P, DH * T], BF, tag="tmp")
            if YA_DVE:
                nc.vector.tensor_copy(out=ya[:], in_=yacc[:])
            else:
                nc.scalar.copy(out=ya[:], in_=yacc[:])
            # transpose y [i,(d-half,t)] back into y_d [d,(i,t)]
            ya3 = ya[:].rearrange("p (d t) -> p d t", t=T)
            for t0 in range(0, T, TG):
                pt = ps.tile([DH, TG * P], BF, tag="tt")
                for k in range(TG):
                    nc.tensor.transpose(pt[:, k * P:(k + 1) * P],
                                        ya3[:, :, t0 + k], identb[:P, :P])
                dst = y_d[d0:d0 + DH, :].rearrange(
                    "p (i t) -> p i t", t=T)[:, :, t0:t0 + TG]
                nc.scalar.copy(out=dst,
                               in_=pt[:].rearrange("p (t i) -> p i t", t=TG))
        while ndone < len(extra):
            extra[ndone]()
            ndone += 1

        # ---- gate: y2 = (y_d + xc2*D) * silu(z) ----
        xc2, z = st["acc"], st["z"]
        sz = sb.tile([DI, CT], BF, tag="du_sz")  # du dead post-transpose
        sg2 = sb.tile([DI, CT], BF, tag="bc")    # bc dead post-transpose
        nc.scalar.activation(sg2[:], z[:], AF.Sigmoid)
        nc.vector.tensor_tensor(sz[:], z[:], sg2[:], OP.mult)
        if use_d:
            nc.vector.scalar_tensor_tensor(
                out=y_d[:], in0=xc2[:], scalar=ctf["D"][:, 0:1],
                in1=y_d[:], op0=OP.mult, op1=OP.add,
            )
        else:
            nc.vector.tensor_tensor(y_d[:], xc2[:], y_d[:], OP.add)
        nc.vector.tensor_tensor(sz[:], y_d[:], sz[:], OP.mult)

        # ---- out = w_out.T @ y2, computed directly transposed: for each
        # n (whose (b,t) slab is contiguous in the i=(n,b) column order),
        # psum[(b t), f] = y2[:, n-slab].T @ w_out. No yo staging and no
        # output transposes. ----
        stg = sb1.tile([bpb * T, n_c * F], FP, tag="ostg")
        NGO = 8                        # n's per out psum tile
        for g in range(n_c // NGO):
            pt = ps.tile([bpb * T, NGO * F], FP, tag="tt")
            for k in range(NGO):
                n_ = g * NGO + k
                nc.tensor.matmul(
                    pt[:, k * F:(k + 1) * F],
                    sz[:, n_ * bpb * T:(n_ + 1) * bpb * T],
                    ct["w_out"][:], start=True, stop=True)
            if blk == nblk - 1 and g % 2 == 1:
                # tail drain: split the last block's staging copies
                nc.vector.tensor_copy(
                    out=stg[:, g * NGO * F:(g + 1) * NGO * F], in_=pt[:])
            else:
                nc.scalar.copy(
                    out=stg[:, g * NGO * F:(g + 1) * NGO * F], in_=pt[:])
        nc.sync.dma_start(
            out_d[b0:b0 + bpb].rearrange("b t n f -> (b t) (n f)"), stg[:])

    units, st = front_units(0)
    for u in units:
        u()
    for blk in range(nblk):
        if blk + 1 < nblk:
            nunits, nst = front_units(blk + 1)
        else:
            nunits, nst = [], None
        scanback(blk, st, nunits)
        st = nst
    es.close()


_CACHE = {}


def _get_program(key, consts, n_c):
    if key not in _CACHE:
        _CACHE[key] = build_program(n_c, consts)
    return _CACHE[key]


def _make_in_maps(inputs, consts):
    x = np.asarray(inputs["x"], np.float32)
    qk = np.asarray(inputs["qk"], np.float32)
    n_c = N // NCORES
    base = {
        "W1xk0": consts["W1xk"][0], "W1xk1": consts["W1xk"][1],
        "W1xk2": consts["W1xk"][2], "W1xk3": consts["W1xk"][3],
        "W1z": np.ascontiguousarray(consts["W1z"]),
        "b1x": consts["b1x"].reshape(DI, 1).copy(),
        "b1z": consts["b1z"].reshape(DI, 1).copy(),
        "W2dt": np.ascontiguousarray(consts["W2dt"]),
        "W2bc": np.ascontiguousarray(consts["W2bc"]),
        "b_dt": consts["b_dt"].reshape(DI, 1).copy(),
        "conv_w": np.ascontiguousarray(consts["conv_w"]),
        "conv_b": consts["conv_b"].reshape(DI, 1).copy(),
        "D": consts["D"].reshape(DI, 1).copy(),
        "w_out": np.ascontiguousarray(consts["w_out"]),
    }
    in_maps = []
    for c in range(NCORES):
        sl = slice(c * n_c, (c + 1) * n_c)
        m = dict(base)
        m["x_sh"] = np.ascontiguousarray(x[:, :, sl, :])
        m["qk_sh"] = np.ascontiguousarray(qk[:, :, sl, :])
        in_maps.append(m)
    return in_maps


def kernel(**inputs):
    consts = _host_consts(inputs)
    n_c = N // NCORES
    nc = _get_program("main", consts, n_c)
    in_maps = _make_in_maps(inputs, consts)
    res = run_bass_kernel_spmd(nc, in_maps, core_ids=list(range(NCORES)))
    out = np.empty((B, T, N, F), np.float32)
    for c in range(NCORES):
        sl = slice(c * n_c, (c + 1) * n_c)
        out[:, :, sl, :] = res.results[c]["out_sh"].reshape(B, T, n_c, F)
    return out
